# revision 12
# baseline (speedup 1.0000x reference)
"""Trainium2 Bass kernel for a DP-GAT layer (dense masked attention).

Computes, for x:[B,N,D], A_shape:[N,N] (0/1 adjacency), q,k,v:[D,D]:
    Q = x@q ; K = x@k
    S = Q @ K^T / sqrt(D)
    W = exp(8*tanh(S/8)) * A_shape
    out = (W / W.sum(-1, keepdims=True)) @ x @ v

Sharding: rows of N split across 8 NeuronCores (1024 rows each), SPMD,
no collectives. Each core computes its row-block of scores in a
flash-attention-style fused loop and writes its row-block of the
output. Host scatters inputs / gathers outputs.

The kernel is ScalarE(ACT)-bound: tanh and exp must each touch every
score element (33.6M per core) and only the ACT engine has
transcendentals, so the floor is ~2 x 262k cycles/partition. The whole
design keeps ACT at ~100% duty:

  - The small projections K^T=(x@k)^T, Q^T, x@v (1.6% of FLOPs) are
    precomputed on the host (fp32, rounded to fp16), freeing the PE and
    DVE from prep work and freeing 2 PSUM banks.
  - The [N, RB] adjacency row-block streams from HBM as fp8-e4m3 (0/1
    is exact): half the fp16 traffic (33.6MB/core vs 67). It streams
    (pool-paced, one 256KB strip per group) rather than sitting
    resident in SBUF: a resident mask's 13MB dependency-free startup
    DMA burst trips the chip's power governor into a low p-state that
    slows EVERY engine ~20-40% for the whole run (measured: 563us ->
    799us). Pool-paced streaming keeps sustained DMA at a gentle
    ~70 GB/s.
  - tanh writes fp16 (ACT rate is dtype-independent; halves SBUF
    traffic), exp is fp16->fp16.
  - PSUM: 4 banks score group + 2x2 banks double-buffered PV
    accumulator, so the end-of-chunk normalize never stalls the next
    chunk. PE start=True clears a full PSUM bank, so each acc's two
    banks are zeroed by two full-bank dummy matmuls and all real PV
    matmuls accumulate with start=False (col 128 = rowsum via ones
    column of xv).

Device-side flow (per core, per batch), groups of 4 key-tiles:
    S^T  = KT_tile^T @ QT_chunk      -> PSUM [128, 4, 512] fp32 (PE)
    u    = tanh(S^T / (8*sqrt(D)))   -> SBUF fp16 (ACT, scale fused)
    w    = exp(8*u)                  -> SBUF fp16 (ACT, scale fused)
    p    = w * mask_tile(fp8)        -> SBUF fp16 (DVE)
    acc[i,0:129] += p_slice^T @ xv   -> PSUM (PE)
    out = acc[:, :128] * (1/acc[:, 128])  -> DMA to DRAM (DVE)
"""

import math
import sys
from contextlib import ExitStack

import numpy as np

try:
    import concourse.bass as bass  # noqa: F401
except ImportError:  # pragma: no cover
    sys.path.insert(0, "/opt/trn_rl_repo")
    import concourse.bass as bass  # noqa: F401

import concourse.mybir as mybir
import concourse.tile as tile
from concourse import bacc
from concourse.bass_utils import run_bass_kernel_spmd

F32 = mybir.dt.float32
F16 = mybir.dt.float16
F8 = mybir.dt.float8e4

B, N, D = 4, 8192, 128
NCORES = 8
RB = N // NCORES  # query rows per core

IC = 512          # query-row chunk (free dim of score matmuls)
NIC = RB // IC    # i-chunks per core
JG = 4            # key 128-tiles per score group
NJT = N // 128    # key tiles total
NG = NJT // JG    # groups per i-chunk
PE_FILL = 10      # keep-warm zero-matmuls per group (see group())


def build_program():
    nc = bacc.Bacc("TRN2", target_bir_lowering=False, debug=False)

    # host-precomputed: kt=(x@k)^T, qt=(x@q)^T row-block, xv=x@v (+ones col)
    kt_d = nc.dram_tensor("kt", [B, D, N], F16, kind="ExternalInput").ap()
    qt_d = nc.dram_tensor("qt", [B, D, RB], F16, kind="ExternalInput").ap()
    # [key-in-tile, key-tile, col] so each partition's DMA run is contiguous
    xv_d = nc.dram_tensor("xv", [B, 128, NJT, 130], F16, kind="ExternalInput").ap()
    # [partition, i-chunk, group, JG*IC]: one contiguous 2KB run per
    # partition per group strip (512B runs measurably aggravate the HBM
    # activity throttle)
    mask_d = nc.dram_tensor("maskT", [128, NIC, NG, JG * IC], F8, kind="ExternalInput").ap()
    out_d = nc.dram_tensor("out", [B, RB, D], F32, kind="ExternalOutput").ap()

    tanh_scale = 1.0 / (8.0 * math.sqrt(float(D)))

    with tile.TileContext(nc) as tc, ExitStack() as ctx:
        consts = ctx.enter_context(tc.tile_pool(name="consts", bufs=1))
        kt_pool = ctx.enter_context(tc.tile_pool(name="kt", bufs=2))
        qt_pool = ctx.enter_context(tc.tile_pool(name="qt", bufs=2))
        xv_pool = ctx.enter_context(tc.tile_pool(name="xv", bufs=2))
        m_pool = ctx.enter_context(tc.tile_pool(name="m", bufs=3))
        u_pool = ctx.enter_context(tc.tile_pool(name="u", bufs=3))
        w_pool = ctx.enter_context(tc.tile_pool(name="w", bufs=3))
        p_pool = ctx.enter_context(tc.tile_pool(name="p", bufs=3))
        ob_pool = ctx.enter_context(tc.tile_pool(name="ob", bufs=4))
        rs_pool = ctx.enter_context(tc.tile_pool(name="rs", bufs=4))
        st_ps = ctx.enter_context(tc.tile_pool(name="st_ps", bufs=1, space="PSUM"))
        acc_ps = ctx.enter_context(tc.tile_pool(name="acc_ps", bufs=2, space="PSUM"))

        zeros = consts.tile([128, 512], F16)
        nc.vector.memset(zeros[:], 0.0)

        tiles = {}  # b -> (kt, qt, xv)

        def load_batch(b):
            """Issue DMAs for batch b's kt/qt/xv (4 strips each so the
            first score group's deps land early)."""
            kt = kt_pool.tile([128, N], F16)
            qt = qt_pool.tile([128, RB], F16)
            xv = xv_pool.tile([128, NJT, 130], F16)
            tiles[b] = (kt, qt, xv)
            nc.sync.dma_start(qt[:], qt_d[b])
            for s in range(4):
                ks = N // 4
                nc.sync.dma_start(
                    kt[:, s * ks : (s + 1) * ks], kt_d[b][:, s * ks : (s + 1) * ks]
                )
                ts = NJT // 4
                nc.sync.dma_start(
                    xv[:, s * ts : (s + 1) * ts, :],
                    xv_d[b][:, s * ts : (s + 1) * ts, :],
                )

        def zero_acc(acc):
            # PE start=True clears the WHOLE PSUM bank, so the two acc
            # slots sharing a bank are zeroed by one full-bank dummy
            # matmul; all real PV matmuls accumulate with start=False.
            for hb in range(2):
                nc.tensor.matmul(
                    acc[:, hb * 512 : (hb + 1) * 512],
                    zeros[:, 0:128], zeros[:],
                    start=True, stop=False, skip_group_check=True,
                )

        def group(b, ic, g, acc):
            kt, qt, xv = tiles[b]
            stp = st_ps.tile([128, JG, IC], F32)
            for j in range(JG):
                nc.tensor.matmul(
                    stp[:, j],
                    kt[:, (g * JG + j) * 128 : (g * JG + j + 1) * 128],
                    qt[:, ic * IC : (ic + 1) * IC],
                    start=True, stop=True,
                )
            if g == 0:
                # placed after the first score matmuls so the PE can issue
                # them immediately at chunk start
                zero_acc(acc)
            # PE keep-warm filler: the PE's DVFS governor down-states the
            # engine when it idles, then runs the next burst at ~half rate
            # (ham k=4 windows). Zero-matmuls into acc's unused gap columns
            # (cols 132..251 of each 256-col slot are never read) keep the
            # PE continuously busy so score/PV matmuls run at full clock.
            for f in range(PE_FILL):
                fb = f & 1
                nc.tensor.matmul(
                    acc[:, fb * 512 + 132 : fb * 512 + 252],
                    zeros[:, 0:128], zeros[:, 0:120],
                    start=False, stop=False, skip_group_check=True,
                )
            u = u_pool.tile([128, JG, IC], F16)
            nc.scalar.activation(
                u[:], stp[:], mybir.ActivationFunctionType.Tanh, scale=tanh_scale
            )
            w = w_pool.tile([128, JG, IC], F16)
            nc.scalar.activation(
                w[:], u[:], mybir.ActivationFunctionType.Exp, scale=8.0
            )
            m = m_pool.tile([128, JG, IC], F8)
            nc.sync.dma_start(m[:].rearrange("p j i -> p (j i)"), mask_d[:, ic, g])
            p = p_pool.tile([128, JG, IC], F16)
            nc.vector.tensor_mul(p[:], w[:], m[:])
            for j in range(JG):
                for s in range(IC // 128):
                    nc.tensor.matmul(
                        acc[:, s * 256 : s * 256 + 129],
                        p[:, j, s * 128 : (s + 1) * 128],
                        xv[:, g * JG + j, 0:129],
                        start=False,
                        stop=(g == NG - 1 and j == JG - 1),
                        skip_group_check=True,
                    )

        load_batch(0)
        for b in range(B):
            for ic in range(NIC):
                if ic == NIC - 1 and b + 1 < B:
                    load_batch(b + 1)
                acc = acc_ps.tile([128, 1024], F32)
                for g in range(NG):
                    group(b, ic, g, acc)
                for s in range(IC // 128):
                    rs = rs_pool.tile([128, 1], F32)
                    nc.vector.reciprocal(rs[:], acc[:, s * 256 + 128 : s * 256 + 129])
                    ob = ob_pool.tile([128, 128], F32)
                    nc.vector.tensor_scalar_mul(
                        ob[:], acc[:, s * 256 : s * 256 + 128], rs[:]
                    )
                    nc.sync.dma_start(
                        out_d[b, ic * IC + s * 128 : ic * IC + (s + 1) * 128, :],
                        ob[:],
                    )

    nc.compile()
    return nc


_CACHED_NC = None


def _get_program():
    global _CACHED_NC
    if _CACHED_NC is None:
        _CACHED_NC = build_program()
    return _CACHED_NC


def make_in_maps(x, A_shape, q, k, v):
    x32 = np.ascontiguousarray(x, dtype=np.float32).reshape(-1, D)
    K = (x32 @ np.asarray(k, np.float32)).reshape(B, N, D)
    Q = (x32 @ np.asarray(q, np.float32)).reshape(B, N, D)
    XV = (x32 @ np.asarray(v, np.float32)).reshape(B, N, D)

    kt = np.ascontiguousarray(K.transpose(0, 2, 1)).astype(np.float16)  # [B,D,N]
    xv = np.zeros((B, N, 130), np.float16)
    xv[:, :, :128] = XV.astype(np.float16)
    xv[:, :, 128] = 1.0
    # [B, key-in-tile, key-tile, col]: contiguous per-partition DMA runs
    xv = np.ascontiguousarray(xv.reshape(B, NJT, 128, 130).transpose(0, 2, 1, 3))

    f8 = np.dtype(mybir.dt.np(F8))
    A32 = np.asarray(A_shape, np.float32)
    in_maps = []
    for c in range(NCORES):
        r0 = c * RB
        qt = np.ascontiguousarray(
            Q[:, r0 : r0 + RB, :].transpose(0, 2, 1)
        ).astype(np.float16)
        # maskT [N, RB] -> [key-in-tile, i-chunk, group, JG*IC]
        maskT = np.ascontiguousarray(
            A32[r0 : r0 + RB, :].T
            .reshape(NG, JG, 128, NIC, IC)
            .transpose(2, 3, 0, 1, 4)
            .reshape(128, NIC, NG, JG * IC)
        ).astype(f8)
        in_maps.append({"kt": kt, "qt": qt, "xv": xv, "maskT": maskT})
    return in_maps


def kernel(x, A_shape, q, k, v):
    nc = _get_program()
    in_maps = make_in_maps(x, A_shape, q, k, v)
    res = run_bass_kernel_spmd(nc, in_maps, list(range(NCORES)))
    out = np.concatenate([res.results[c]["out"] for c in range(NCORES)], axis=1)
    return out.astype(np.float32)


# revision 43
# speedup vs baseline: 1.4567x; 1.4567x over previous
"""Trainium2 Bass kernel for a DP-GAT layer (dense masked attention).

Computes, for x:[B,N,D], A_shape:[N,N] (0/1 adjacency), q,k,v:[D,D]:
    Q = x@q ; K = x@k
    S = Q @ K^T / sqrt(D)
    W = exp(8*tanh(S/8)) * A_shape
    out = (W / W.sum(-1, keepdims=True)) @ x @ v

Sharding: rows of N split across 8 NeuronCores (1024 rows each), SPMD,
no collectives. Each core streams its row-block of the mask, computes
scores in a flash-attention-style fused loop, and writes its row-block
of the output. Host scatters inputs / gathers outputs.

Numerics: q,k are split on the host into fp16 hi+lo pairs; K^T and Q^T
are computed as two-pass fp16 matmuls with fp32 PSUM accumulation and
stored as fp16. fp16 score operands keep the final output within ~2e-3
relative of the fp32 reference (fp16 matmuls run at full PE rate, and
the exp(8*tanh) amplification of coarser dtypes is unacceptable).

Device-side flow (per core, per batch):
    KT  = k^T @ x^T  (fp16 2-pass)   [D, N]
    QT  = q^T @ xrows^T (fp16 2-pass)[D, RB]
    xv  = x @ v (+ ones col)         [N, D+1] fp16
    per i-chunk of 512 query rows:
      per group of 4 key-tiles (512 keys):
        S^T  = KT_tile^T @ QT_chunk      -> PSUM [128, 4, 512] fp32
        u    = tanh(S^T / (8*sqrt(D)))   -> SBUF fp32  (ScalarE, scale fused)
        w    = exp(8*u)                  -> SBUF fp16  (ScalarE, scale fused)
        p    = w * maskT_tile            -> SBUF fp16  (VectorE)
        acc[i,0:129] += p_slice^T @ xv   -> PSUM       (fp16 matmuls; col 128
                                                        = rowsum via ones col)
      out = acc[:, :128] * (1/acc[:, 128])  -> DMA to DRAM

The kernel is ScalarE(ACT)-bound: tanh and exp must each touch every
score element (33.6M per core) and only the ACT engine has
transcendentals, so the floor is ~2 x 262k cycles/partition (~500us).
The main loop is a 3-deep software pipeline keeping ACT at ~98% duty
(see the comment in the loop): tanh_g leads its period so the next
group's score matmuls (WAR on the single-buffered score PSUM) release
~2us before tanh_{g+1} needs them, and the PV stage lags 3 iterations
so its DVE-produced operand is always ready and the end-of-chunk
normalize overlaps the next chunk.

The per-batch prep (KT/QT/xv) is interleaved between groups (batch b+1
prep inside batch b's last i-chunk; batch 0's prep between its own
first-i-chunk groups). Besides overlap, the prep matmuls and the fp16
mask streaming keep the PE/DMA demand high enough that the chip's
demand-following DVFS governor (ham k=4 windows, visible in the NTFF
profile) holds full clocks: variants that precomputed the projections
on the host and/or shrank mask traffic to fp8 dropped the PE (and
sometimes ACT) into a ~half-rate state and ran 100-250us SLOWER despite
doing less work.

PSUM bank budget (8 banks of 2KB): score group 4 + PV accumulator 2 +
prep 2. PE matmuls with start=True clear their entire output PSUM bank,
so the two acc slots sharing a bank are zeroed by one full-bank dummy
matmul and all PV matmuls accumulate with start=False. The accumulator
is single-buffered but copied to SBUF in one DVE op before the
normalize, so it frees within ~1 period of its last PV.
"""

import math
import sys
from contextlib import ExitStack

import numpy as np

try:
    import concourse.bass as bass  # noqa: F401
except ImportError:  # pragma: no cover
    sys.path.insert(0, "/opt/trn_rl_repo")
    import concourse.bass as bass  # noqa: F401

import concourse.mybir as mybir
import concourse.tile as tile
from concourse import bacc
from concourse.bass_utils import run_bass_kernel_spmd

F32 = mybir.dt.float32
F16 = mybir.dt.float16

B, N, D = 4, 8192, 128
NCORES = 8
RB = N // NCORES  # query rows per core

IC = 512          # query-row chunk (free dim of score matmuls)
NIC = RB // IC    # i-chunks per core
JG = 4            # key 128-tiles per score group
NJT = N // 128    # key tiles total
NG = NJT // JG    # groups per i-chunk
CH = JG * 128     # xt prep chunk width (chunk g produces what group g consumes)


def build_program():
    nc = bacc.Bacc("TRN2", target_bir_lowering=False, debug=False)

    xt = nc.dram_tensor("xt", [B, D, N], F16, kind="ExternalInput").ap()
    xqt = nc.dram_tensor("xqt", [B, D, RB], F16, kind="ExternalInput").ap()
    # host-precomputed batch-0 boot tensors: qt plus kt/xv for key chunks
    # 0-1, so the first score group starts ~11us earlier (device prep's
    # serial xq-DMA -> QT-matmul -> copy chain is skipped for batch 0)
    bqt_d = nc.dram_tensor("boot_qt", [D, RB], F16, kind="ExternalInput").ap()
    bkt_d = nc.dram_tensor("boot_kt", [D, 2 * CH], F16, kind="ExternalInput").ap()
    bxv_d = nc.dram_tensor("boot_xv", [128, 2 * CH // 128, 130], F16, kind="ExternalInput").ap()
    maskT = nc.dram_tensor("maskT", [N, RB], F16, kind="ExternalInput").ap()
    qh_d = nc.dram_tensor("q_hi", [D, D], F16, kind="ExternalInput").ap()
    ql_d = nc.dram_tensor("q_lo", [D, D], F16, kind="ExternalInput").ap()
    kh_d = nc.dram_tensor("k_hi", [D, D], F16, kind="ExternalInput").ap()
    kl_d = nc.dram_tensor("k_lo", [D, D], F16, kind="ExternalInput").ap()
    v_d = nc.dram_tensor("v", [D, D], F16, kind="ExternalInput").ap()
    out_d = nc.dram_tensor("out", [B, RB, D], F32, kind="ExternalOutput").ap()

    # [128, key-tile, query-col] view of the transposed mask block
    maskT_r = maskT.rearrange("(t p) i -> p t i", p=128)

    tanh_scale = 1.0 / (8.0 * math.sqrt(float(D)))

    with tile.TileContext(nc) as tc, ExitStack() as ctx:
        consts = ctx.enter_context(tc.tile_pool(name="consts", bufs=1))
        kt_pool = ctx.enter_context(tc.tile_pool(name="kt", bufs=2))
        qt_pool = ctx.enter_context(tc.tile_pool(name="qt", bufs=2))
        xv_pool = ctx.enter_context(tc.tile_pool(name="xv", bufs=2))
        xc_pool = ctx.enter_context(tc.tile_pool(name="xc", bufs=3))
        m_pool = ctx.enter_context(tc.tile_pool(name="m", bufs=4))
        u_pool = ctx.enter_context(tc.tile_pool(name="u", bufs=3))
        w_pool = ctx.enter_context(tc.tile_pool(name="w", bufs=3))
        p_pool = ctx.enter_context(tc.tile_pool(name="p", bufs=4))
        ac_pool = ctx.enter_context(tc.tile_pool(name="ac", bufs=2))
        ob_pool = ctx.enter_context(tc.tile_pool(name="ob", bufs=4))
        rs_pool = ctx.enter_context(tc.tile_pool(name="rs", bufs=4))
        prep_ps = ctx.enter_context(tc.tile_pool(name="prep_ps", bufs=2, space="PSUM"))
        st_ps = ctx.enter_context(tc.tile_pool(name="st_ps", bufs=1, space="PSUM"))
        acc_ps = ctx.enter_context(tc.tile_pool(name="acc_ps", bufs=1, space="PSUM"))

        zeros = consts.tile([128, 512], F16)
        nc.vector.memset(zeros[:], 0.0)
        qh_sb = consts.tile([D, D], F16)
        nc.sync.dma_start(qh_sb[:], qh_d[:])
        ql_sb = consts.tile([D, D], F16)
        nc.sync.dma_start(ql_sb[:], ql_d[:])
        kh_sb = consts.tile([D, D], F16)
        nc.sync.dma_start(kh_sb[:], kh_d[:])
        kl_sb = consts.tile([D, D], F16)
        nc.sync.dma_start(kl_sb[:], kl_d[:])
        v_sb = consts.tile([D, D], F16)
        nc.sync.dma_start(v_sb[:], v_d[:])

        tiles = {}  # b -> (kt, qt, xv)

        def boot_head():
            """Batch-0 tiles come from the host boot tensors: qt whole,
            kt/xv chunks 0-1 by DMA; ones column only for tiles 8+."""
            kt = kt_pool.tile([128, N], F16)
            qt = qt_pool.tile([128, RB], F16)
            xv = xv_pool.tile([128, NJT, 130], F16)
            tiles[0] = (kt, qt, xv)
            nc.sync.dma_start(qt[:], bqt_d[:])
            nc.sync.dma_start(kt[:, 0 : 2 * CH], bkt_d[:])
            nc.sync.dma_start(xv[:, 0 : 2 * CH // 128, :], bxv_d[:])
            nc.vector.memset(xv[:, 2 * CH // 128 :, 128:129], 1.0)

        def prep_head(b):
            """Allocate batch-b tiles; compute QT; set xv ones column."""
            kt = kt_pool.tile([128, N], F16)
            qt = qt_pool.tile([128, RB], F16)
            xv = xv_pool.tile([128, NJT, 130], F16)
            tiles[b] = (kt, qt, xv)
            nc.vector.memset(xv[:, :, 128:129], 1.0)
            xq = qt_pool.tile([128, RB], F16, tag="xq")
            nc.sync.dma_start(xq[:], xqt[b])
            qch = min(CH, RB)
            for c in range(RB // qch):
                pq = prep_ps.tile([128, qch], F32, tag="prep")
                nc.tensor.matmul(
                    pq[:], qh_sb[:], xq[:, c * qch : (c + 1) * qch],
                    start=True, stop=False,
                )
                nc.tensor.matmul(
                    pq[:], ql_sb[:], xq[:, c * qch : (c + 1) * qch],
                    start=False, stop=True,
                )
                nc.vector.tensor_copy(qt[:, c * qch : (c + 1) * qch], pq[:])

        def prep_chunk(b, c):
            """Compute kt columns and xv tiles for chunk c of batch b."""
            kt, _, xv = tiles[b]
            xc = xc_pool.tile([128, CH], F16)
            nc.sync.dma_start(xc[:], xt[b][:, c * CH : (c + 1) * CH])
            pk = prep_ps.tile([128, CH], F32, tag="prep")
            nc.tensor.matmul(pk[:], kh_sb[:], xc[:], start=True, stop=False)
            nc.tensor.matmul(pk[:], kl_sb[:], xc[:], start=False, stop=True)
            nc.vector.tensor_copy(kt[:, c * CH : (c + 1) * CH], pk[:])
            for s in range(CH // 128):
                pxv = prep_ps.tile([128, 128], F32, tag="prep")
                nc.tensor.matmul(
                    pxv[:], xc[:, s * 128 : (s + 1) * 128], v_sb[:],
                    start=True, stop=True,
                )
                nc.vector.tensor_copy(xv[:, c * (CH // 128) + s, 0:128], pxv[:])

        def zero_acc(acc):
            # PE start=True clears the WHOLE PSUM bank, so the two acc
            # slots sharing a bank are zeroed by one full-bank dummy
            # matmul; all real PV matmuls accumulate with start=False.
            for hb in range(2):
                nc.tensor.matmul(
                    acc[:, hb * 512 : (hb + 1) * 512],
                    zeros[:, 0:128], zeros[:],
                    start=True, stop=False, skip_group_check=True,
                )

        # ---- 3-deep software-pipelined main loop --------------------
        # stage 1 (iter gi):   score matmuls + tanh  (+ prep attachments)
        # stage 2 (iter gi+1): exp + mask multiply
        # stage 3 (iter gi+3): PV matmuls (+ zero/normalize at chunk edges)
        #
        # tanh_g is the FIRST ScalarE op of its period, so the next
        # group's score matmuls (WAR on the single-buffered score PSUM)
        # are released ~2us before tanh_{g+1} needs them, and the PE is
        # never on the ScalarE critical path. The PV stage lags by 3 so
        # its operand (p, produced by DVE ~2.3us after exp ends) is
        # always ready before the PE reaches it. The lag also slides the
        # end-of-chunk normalize into the next chunk's first iterations,
        # where it overlaps the pipeline instead of stalling it.

        SEQ = [(b, ic, g) for b in range(B) for ic in range(NIC) for g in range(NG)]
        PVLAG = 3
        st1, st2, accs = {}, {}, {}

        boot_head()

        for gi in range(len(SEQ) + PVLAG):
            if gi < len(SEQ):
                b, ic, g = SEQ[gi]
                if ic == NIC - 1 and g == 0 and b + 1 < B:
                    prep_head(b + 1)
                if b == 0 and ic == 0 and g + 2 < NG:
                    prep_chunk(0, g + 2)
                if ic == NIC - 1 and b + 1 < B:
                    prep_chunk(b + 1, g)
                kt, qt, xv = tiles[b]
                stp = st_ps.tile([128, JG, IC], F32)
                for j in range(JG):
                    nc.tensor.matmul(
                        stp[:, j],
                        kt[:, (g * JG + j) * 128 : (g * JG + j + 1) * 128],
                        qt[:, ic * IC : (ic + 1) * IC],
                        start=True, stop=True,
                    )
                m = m_pool.tile([128, JG, IC], F16)
                nc.sync.dma_start(
                    m[:], maskT_r[:, g * JG : (g + 1) * JG, ic * IC : (ic + 1) * IC]
                )
                u = u_pool.tile([128, JG, IC], F32)
                nc.scalar.activation(
                    u[:], stp[:], mybir.ActivationFunctionType.Tanh, scale=tanh_scale
                )
                st1[gi] = (u, m)

            j2 = gi - 1
            if 0 <= j2 < len(SEQ):
                u, m = st1.pop(j2)
                w = w_pool.tile([128, JG, IC], F16)
                nc.scalar.activation(
                    w[:], u[:], mybir.ActivationFunctionType.Exp, scale=8.0
                )
                p = p_pool.tile([128, JG, IC], F16)
                nc.vector.tensor_mul(p[:], w[:], m[:])
                st2[j2] = p

            j3 = gi - PVLAG
            if 0 <= j3 < len(SEQ):
                b, ic, g = SEQ[j3]
                if g == 0:
                    acc = acc_ps.tile([128, 1024], F32)
                    accs[(b, ic)] = acc
                    zero_acc(acc)
                acc = accs[(b, ic)]
                p = st2.pop(j3)
                xv = tiles[b][2]
                for j in range(JG):
                    for s in range(IC // 128):
                        nc.tensor.matmul(
                            acc[:, s * 256 : s * 256 + 129],
                            p[:, j, s * 128 : (s + 1) * 128],
                            xv[:, g * JG + j, 0:129],
                            start=False,
                            stop=(g == NG - 1 and j == JG - 1),
                            skip_group_check=True,
                        )
                if g == NG - 1:
                    # copy acc out of PSUM in one op so the single
                    # accumulator buffer frees quickly, then normalize
                    # from the SBUF copy: one strided reciprocal over the
                    # 4 rowsum columns, and one DMA for all 512 out rows
                    ac = ac_pool.tile([128, 1024], F32)
                    nc.vector.tensor_copy(ac[:], acc[:])
                    del accs[(b, ic)]
                    acr = ac[:].rearrange("p (s c) -> p s c", s=IC // 128)
                    rs = rs_pool.tile([128, IC // 128], F32)
                    nc.vector.reciprocal(rs[:], acr[:, :, 128])
                    ob = ob_pool.tile([128, IC // 128, 128], F32)
                    for s in range(IC // 128):
                        nc.vector.tensor_scalar_mul(
                            ob[:, s], acr[:, s, 0:128], rs[:, s : s + 1]
                        )
                    nc.sync.dma_start(
                        out_d[b][ic * IC : (ic + 1) * IC, :].rearrange(
                            "(s q) d -> q s d", q=128
                        ),
                        ob[:],
                    )

    nc.compile()
    return nc


_CACHED_NC = None


def _get_program():
    global _CACHED_NC
    if _CACHED_NC is None:
        _CACHED_NC = build_program()
    return _CACHED_NC


def _split16(a):
    hi = a.astype(np.float16)
    lo = (a - hi.astype(np.float32)).astype(np.float16)
    return hi, lo


def make_in_maps(x, A_shape, q, k, v):
    x = np.ascontiguousarray(x, dtype=np.float32)
    xt = np.ascontiguousarray(x.transpose(0, 2, 1)).astype(np.float16)  # [B, D, N]
    q_hi, q_lo = _split16(np.ascontiguousarray(q, dtype=np.float32))
    k_hi, k_lo = _split16(np.ascontiguousarray(k, dtype=np.float32))
    v16 = np.ascontiguousarray(v, dtype=np.float32).astype(np.float16)
    q32 = np.asarray(q, np.float32)
    # batch-0 boot tensors (see build_program): kt/xv for keys 0:2*CH
    boot_kt = np.ascontiguousarray(
        (x[0, : 2 * CH, :] @ np.asarray(k, np.float32)).T
    ).astype(np.float16)
    bxv = np.zeros((2 * CH, 130), np.float16)
    bxv[:, :128] = (x[0, : 2 * CH, :] @ np.asarray(v, np.float32)).astype(np.float16)
    bxv[:, 128] = 1.0
    boot_xv = np.ascontiguousarray(
        bxv.reshape(2 * CH // 128, 128, 130).transpose(1, 0, 2)
    )
    in_maps = []
    for c in range(NCORES):
        r0 = c * RB
        xqt = np.ascontiguousarray(
            x[:, r0 : r0 + RB, :].transpose(0, 2, 1)
        ).astype(np.float16)
        maskT = np.ascontiguousarray(A_shape[r0 : r0 + RB, :].T, dtype=np.float16)
        boot_qt = np.ascontiguousarray(
            (x[0, r0 : r0 + RB, :] @ q32).T
        ).astype(np.float16)
        in_maps.append(
            {
                "xt": xt,
                "xqt": xqt,
                "maskT": maskT,
                "q_hi": q_hi,
                "q_lo": q_lo,
                "k_hi": k_hi,
                "k_lo": k_lo,
                "v": v16,
                "boot_qt": boot_qt,
                "boot_kt": boot_kt,
                "boot_xv": boot_xv,
            }
        )
    return in_maps


def kernel(x, A_shape, q, k, v):
    nc = _get_program()
    in_maps = make_in_maps(x, A_shape, q, k, v)
    res = run_bass_kernel_spmd(nc, in_maps, list(range(NCORES)))
    out = np.concatenate([res.results[c]["out"] for c in range(NCORES)], axis=1)
    return out.astype(np.float32)



# revision 46
# speedup vs baseline: 1.4683x; 1.0080x over previous
"""Trainium2 Bass kernel for a DP-GAT layer (dense masked attention).

Computes, for x:[B,N,D], A_shape:[N,N] (0/1 adjacency), q,k,v:[D,D]:
    Q = x@q ; K = x@k
    S = Q @ K^T / sqrt(D)
    W = exp(8*tanh(S/8)) * A_shape
    out = (W / W.sum(-1, keepdims=True)) @ x @ v

Sharding: rows of N split across 8 NeuronCores (1024 rows each), SPMD,
no collectives. Each core streams its row-block of the mask, computes
scores in a flash-attention-style fused loop, and writes its row-block
of the output. Host scatters inputs / gathers outputs.

Numerics: q,k are split on the host into fp16 hi+lo pairs; K^T and Q^T
are computed as two-pass fp16 matmuls with fp32 PSUM accumulation and
stored as fp16. fp16 score operands keep the final output within ~2e-3
relative of the fp32 reference (fp16 matmuls run at full PE rate, and
the exp(8*tanh) amplification of coarser dtypes is unacceptable).

Device-side flow (per core, per batch):
    KT  = k^T @ x^T  (fp16 2-pass)   [D, N]
    QT  = q^T @ xrows^T (fp16 2-pass)[D, RB]
    xv  = x @ v (+ ones col)         [N, D+1] fp16
    per i-chunk of 512 query rows:
      per group of 4 key-tiles (512 keys):
        S^T  = KT_tile^T @ QT_chunk      -> PSUM [128, 4, 512] fp32
        u    = tanh(S^T / (8*sqrt(D)))   -> SBUF fp32  (ScalarE, scale fused)
        w    = exp(8*u)                  -> SBUF fp16  (ScalarE, scale fused)
        p    = w * maskT_tile            -> SBUF fp16  (VectorE)
        acc[i,0:129] += p_slice^T @ xv   -> PSUM       (fp16 matmuls; col 128
                                                        = rowsum via ones col)
      out = acc[:, :128] * (1/acc[:, 128])  -> DMA to DRAM

The kernel is ScalarE(ACT)-bound: tanh and exp must each touch every
score element (33.6M per core) and only the ACT engine has
transcendentals, so the floor is ~2 x 262k cycles/partition (~500us).
The main loop is a 3-deep software pipeline keeping ACT at ~98% duty
(see the comment in the loop): tanh_g leads its period so the next
group's score matmuls (WAR on the single-buffered score PSUM) release
~2us before tanh_{g+1} needs them, and the PV stage lags 3 iterations
so its DVE-produced operand is always ready and the end-of-chunk
normalize overlaps the next chunk.

The per-batch prep (KT/QT/xv) is interleaved between groups (batch b+1
prep inside batch b's last i-chunk; batch 0's prep between its own
first-i-chunk groups). Besides overlap, the prep matmuls and the fp16
mask streaming keep the PE/DMA demand high enough that the chip's
demand-following DVFS governor (ham k=4 windows, visible in the NTFF
profile) holds full clocks: variants that precomputed the projections
on the host and/or shrank mask traffic to fp8 dropped the PE (and
sometimes ACT) into a ~half-rate state and ran 100-250us SLOWER despite
doing less work.

PSUM bank budget (8 banks of 2KB): score group 4 + PV accumulator 2 +
prep 2. PE matmuls with start=True clear their entire output PSUM bank,
so the two acc slots sharing a bank are zeroed by one full-bank dummy
matmul and all PV matmuls accumulate with start=False. The accumulator
is single-buffered but copied to SBUF in one DVE op before the
normalize, so it frees within ~1 period of its last PV.
"""

import math
import sys
from contextlib import ExitStack

import numpy as np

try:
    import concourse.bass as bass  # noqa: F401
except ImportError:  # pragma: no cover
    sys.path.insert(0, "/opt/trn_rl_repo")
    import concourse.bass as bass  # noqa: F401

import concourse.mybir as mybir
import concourse.tile as tile
from concourse import bacc
from concourse.bass_utils import run_bass_kernel_spmd

F32 = mybir.dt.float32
F16 = mybir.dt.float16

B, N, D = 4, 8192, 128
NCORES = 8
RB = N // NCORES  # query rows per core

IC = 512          # query-row chunk (free dim of score matmuls)
NIC = RB // IC    # i-chunks per core
JG = 4            # key 128-tiles per score group
NJT = N // 128    # key tiles total
NG = NJT // JG    # groups per i-chunk
CH = JG * 128     # xt prep chunk width (chunk g produces what group g consumes)


def build_program():
    nc = bacc.Bacc("TRN2", target_bir_lowering=False, debug=False)

    xt = nc.dram_tensor("xt", [B, D, N], F16, kind="ExternalInput").ap()
    xqt = nc.dram_tensor("xqt", [B, D, RB], F16, kind="ExternalInput").ap()
    # host-precomputed batch-0 boot tensors: qt plus kt/xv for key chunks
    # 0-1, so the first score group starts ~11us earlier (device prep's
    # serial xq-DMA -> QT-matmul -> copy chain is skipped for batch 0)
    bqt_d = nc.dram_tensor("boot_qt", [D, RB], F16, kind="ExternalInput").ap()
    bkt_d = nc.dram_tensor("boot_kt", [D, 2 * CH], F16, kind="ExternalInput").ap()
    bxv_d = nc.dram_tensor("boot_xv", [128, 2 * CH // 128, 130], F16, kind="ExternalInput").ap()
    maskT = nc.dram_tensor("maskT", [N, RB], F16, kind="ExternalInput").ap()
    qh_d = nc.dram_tensor("q_hi", [D, D], F16, kind="ExternalInput").ap()
    ql_d = nc.dram_tensor("q_lo", [D, D], F16, kind="ExternalInput").ap()
    kh_d = nc.dram_tensor("k_hi", [D, D], F16, kind="ExternalInput").ap()
    kl_d = nc.dram_tensor("k_lo", [D, D], F16, kind="ExternalInput").ap()
    v_d = nc.dram_tensor("v", [D, D], F16, kind="ExternalInput").ap()
    out_d = nc.dram_tensor("out", [B, RB, D], F32, kind="ExternalOutput").ap()

    # [128, key-tile, query-col] view of the transposed mask block
    maskT_r = maskT.rearrange("(t p) i -> p t i", p=128)

    tanh_scale = 1.0 / (8.0 * math.sqrt(float(D)))

    with tile.TileContext(nc) as tc, ExitStack() as ctx:
        consts = ctx.enter_context(tc.tile_pool(name="consts", bufs=1))
        kt_pool = ctx.enter_context(tc.tile_pool(name="kt", bufs=2))
        qt_pool = ctx.enter_context(tc.tile_pool(name="qt", bufs=2))
        xv_pool = ctx.enter_context(tc.tile_pool(name="xv", bufs=2))
        xc_pool = ctx.enter_context(tc.tile_pool(name="xc", bufs=3))
        m_pool = ctx.enter_context(tc.tile_pool(name="m", bufs=4))
        u_pool = ctx.enter_context(tc.tile_pool(name="u", bufs=3))
        w_pool = ctx.enter_context(tc.tile_pool(name="w", bufs=3))
        p_pool = ctx.enter_context(tc.tile_pool(name="p", bufs=4))
        ac_pool = ctx.enter_context(tc.tile_pool(name="ac", bufs=2))
        ob_pool = ctx.enter_context(tc.tile_pool(name="ob", bufs=4))
        rs_pool = ctx.enter_context(tc.tile_pool(name="rs", bufs=4))
        prep_ps = ctx.enter_context(tc.tile_pool(name="prep_ps", bufs=2, space="PSUM"))
        st_ps = ctx.enter_context(tc.tile_pool(name="st_ps", bufs=1, space="PSUM"))
        acc_ps = ctx.enter_context(tc.tile_pool(name="acc_ps", bufs=1, space="PSUM"))

        tiles = {}  # b -> (kt, qt, xv)

        def boot_head():
            """Batch-0 tiles come from the host boot tensors: qt whole,
            kt/xv chunks 0-1 by DMA; ones column only for tiles 8+."""
            kt = kt_pool.tile([128, N], F16)
            qt = qt_pool.tile([128, RB], F16)
            xv = xv_pool.tile([128, NJT, 130], F16)
            tiles[0] = (kt, qt, xv)
            nc.sync.dma_start(qt[:], bqt_d[:])
            nc.sync.dma_start(kt[:, 0 : 2 * CH], bkt_d[:])
            nc.sync.dma_start(xv[:, 0 : 2 * CH // 128, :], bxv_d[:])
            nc.vector.memset(xv[:, 2 * CH // 128 :, 128:129], 1.0)

        def prep_head(b):
            """Allocate batch-b tiles; compute QT; set xv ones column."""
            kt = kt_pool.tile([128, N], F16)
            qt = qt_pool.tile([128, RB], F16)
            xv = xv_pool.tile([128, NJT, 130], F16)
            tiles[b] = (kt, qt, xv)
            nc.vector.memset(xv[:, :, 128:129], 1.0)
            xq = qt_pool.tile([128, RB], F16, tag="xq")
            nc.sync.dma_start(xq[:], xqt[b])
            qch = min(CH, RB)
            for c in range(RB // qch):
                pq = prep_ps.tile([128, qch], F32, tag="prep")
                nc.tensor.matmul(
                    pq[:], qh_sb[:], xq[:, c * qch : (c + 1) * qch],
                    start=True, stop=False,
                )
                nc.tensor.matmul(
                    pq[:], ql_sb[:], xq[:, c * qch : (c + 1) * qch],
                    start=False, stop=True,
                )
                nc.vector.tensor_copy(qt[:, c * qch : (c + 1) * qch], pq[:])

        def prep_chunk(b, c):
            """Compute kt columns and xv tiles for chunk c of batch b."""
            kt, _, xv = tiles[b]
            xc = xc_pool.tile([128, CH], F16)
            nc.sync.dma_start(xc[:], xt[b][:, c * CH : (c + 1) * CH])
            pk = prep_ps.tile([128, CH], F32, tag="prep")
            nc.tensor.matmul(pk[:], kh_sb[:], xc[:], start=True, stop=False)
            nc.tensor.matmul(pk[:], kl_sb[:], xc[:], start=False, stop=True)
            nc.vector.tensor_copy(kt[:, c * CH : (c + 1) * CH], pk[:])
            for s in range(CH // 128):
                pxv = prep_ps.tile([128, 128], F32, tag="prep")
                nc.tensor.matmul(
                    pxv[:], xc[:, s * 128 : (s + 1) * 128], v_sb[:],
                    start=True, stop=True,
                )
                nc.vector.tensor_copy(xv[:, c * (CH // 128) + s, 0:128], pxv[:])

        def zero_acc(acc):
            # PE start=True clears the WHOLE PSUM bank, so the two acc
            # slots sharing a bank are zeroed by one full-bank dummy
            # matmul; all real PV matmuls accumulate with start=False.
            for hb in range(2):
                nc.tensor.matmul(
                    acc[:, hb * 512 : (hb + 1) * 512],
                    zeros[:, 0:128], zeros[:],
                    start=True, stop=False, skip_group_check=True,
                )

        # ---- 3-deep software-pipelined main loop --------------------
        # stage 1 (iter gi):   score matmuls + tanh  (+ prep attachments)
        # stage 2 (iter gi+1): exp + mask multiply
        # stage 3 (iter gi+3): PV matmuls (+ zero/normalize at chunk edges)
        #
        # tanh_g is the FIRST ScalarE op of its period, so the next
        # group's score matmuls (WAR on the single-buffered score PSUM)
        # are released ~2us before tanh_{g+1} needs them, and the PE is
        # never on the ScalarE critical path. The PV stage lags by 3 so
        # its operand (p, produced by DVE ~2.3us after exp ends) is
        # always ready before the PE reaches it. The lag also slides the
        # end-of-chunk normalize into the next chunk's first iterations,
        # where it overlaps the pipeline instead of stalling it.

        SEQ = [(b, ic, g) for b in range(B) for ic in range(NIC) for g in range(NG)]
        PVLAG = 3
        st1, st2, accs = {}, {}, {}

        # boot DMAs first: the consts/zeros below are needed only by the
        # prep of later chunks/batches, and each dma_start costs ~0.6us
        # of Sync-engine issue time ahead of the first score group
        boot_head()
        zeros = consts.tile([128, 512], F16)
        nc.vector.memset(zeros[:], 0.0)
        qh_sb = consts.tile([D, D], F16)
        nc.sync.dma_start(qh_sb[:], qh_d[:])
        ql_sb = consts.tile([D, D], F16)
        nc.sync.dma_start(ql_sb[:], ql_d[:])
        kh_sb = consts.tile([D, D], F16)
        nc.sync.dma_start(kh_sb[:], kh_d[:])
        kl_sb = consts.tile([D, D], F16)
        nc.sync.dma_start(kl_sb[:], kl_d[:])
        v_sb = consts.tile([D, D], F16)
        nc.sync.dma_start(v_sb[:], v_d[:])

        for gi in range(len(SEQ) + PVLAG):
            if gi < len(SEQ):
                b, ic, g = SEQ[gi]
                if ic == NIC - 1 and g == 0 and b + 1 < B:
                    prep_head(b + 1)
                if b == 0 and ic == 0 and g + 2 < NG:
                    prep_chunk(0, g + 2)
                if ic == NIC - 1 and b + 1 < B:
                    prep_chunk(b + 1, g)
                kt, qt, xv = tiles[b]
                stp = st_ps.tile([128, JG, IC], F32)
                for j in range(JG):
                    nc.tensor.matmul(
                        stp[:, j],
                        kt[:, (g * JG + j) * 128 : (g * JG + j + 1) * 128],
                        qt[:, ic * IC : (ic + 1) * IC],
                        start=True, stop=True,
                    )
                m = m_pool.tile([128, JG, IC], F16)
                nc.sync.dma_start(
                    m[:], maskT_r[:, g * JG : (g + 1) * JG, ic * IC : (ic + 1) * IC]
                )
                u = u_pool.tile([128, JG, IC], F32)
                nc.scalar.activation(
                    u[:], stp[:], mybir.ActivationFunctionType.Tanh, scale=tanh_scale
                )
                st1[gi] = (u, m)

            j2 = gi - 1
            if 0 <= j2 < len(SEQ):
                u, m = st1.pop(j2)
                w = w_pool.tile([128, JG, IC], F16)
                nc.scalar.activation(
                    w[:], u[:], mybir.ActivationFunctionType.Exp, scale=8.0
                )
                p = p_pool.tile([128, JG, IC], F16)
                nc.vector.tensor_mul(p[:], w[:], m[:])
                st2[j2] = p

            j3 = gi - PVLAG
            if 0 <= j3 < len(SEQ):
                b, ic, g = SEQ[j3]
                if g == 0:
                    acc = acc_ps.tile([128, 1024], F32)
                    accs[(b, ic)] = acc
                    zero_acc(acc)
                acc = accs[(b, ic)]
                p = st2.pop(j3)
                xv = tiles[b][2]
                for j in range(JG):
                    for s in range(IC // 128):
                        nc.tensor.matmul(
                            acc[:, s * 256 : s * 256 + 129],
                            p[:, j, s * 128 : (s + 1) * 128],
                            xv[:, g * JG + j, 0:129],
                            start=False,
                            stop=(g == NG - 1 and j == JG - 1),
                            skip_group_check=True,
                        )
                if g == NG - 1:
                    # copy acc out of PSUM in one op so the single
                    # accumulator buffer frees quickly, then normalize
                    # from the SBUF copy: one strided reciprocal over the
                    # 4 rowsum columns, and one DMA for all 512 out rows.
                    # The last chunk has no successor waiting on acc, so
                    # it normalizes from PSUM directly.
                    if b == B - 1 and ic == NIC - 1:
                        ac = acc
                    else:
                        ac = ac_pool.tile([128, 1024], F32)
                        nc.vector.tensor_copy(ac[:], acc[:])
                    del accs[(b, ic)]
                    acr = ac[:].rearrange("p (s c) -> p s c", s=IC // 128)
                    rs = rs_pool.tile([128, IC // 128], F32)
                    nc.vector.reciprocal(rs[:], acr[:, :, 128])
                    ob = ob_pool.tile([128, IC // 128, 128], F32)
                    for s in range(IC // 128):
                        nc.vector.tensor_scalar_mul(
                            ob[:, s], acr[:, s, 0:128], rs[:, s : s + 1]
                        )
                    nc.sync.dma_start(
                        out_d[b][ic * IC : (ic + 1) * IC, :].rearrange(
                            "(s q) d -> q s d", q=128
                        ),
                        ob[:],
                    )

    nc.compile()
    return nc


_CACHED_NC = None


def _get_program():
    global _CACHED_NC
    if _CACHED_NC is None:
        _CACHED_NC = build_program()
    return _CACHED_NC


def _split16(a):
    hi = a.astype(np.float16)
    lo = (a - hi.astype(np.float32)).astype(np.float16)
    return hi, lo


def make_in_maps(x, A_shape, q, k, v):
    x = np.ascontiguousarray(x, dtype=np.float32)
    xt = np.ascontiguousarray(x.transpose(0, 2, 1)).astype(np.float16)  # [B, D, N]
    q_hi, q_lo = _split16(np.ascontiguousarray(q, dtype=np.float32))
    k_hi, k_lo = _split16(np.ascontiguousarray(k, dtype=np.float32))
    v16 = np.ascontiguousarray(v, dtype=np.float32).astype(np.float16)
    q32 = np.asarray(q, np.float32)
    # batch-0 boot tensors (see build_program): kt/xv for keys 0:2*CH
    boot_kt = np.ascontiguousarray(
        (x[0, : 2 * CH, :] @ np.asarray(k, np.float32)).T
    ).astype(np.float16)
    bxv = np.zeros((2 * CH, 130), np.float16)
    bxv[:, :128] = (x[0, : 2 * CH, :] @ np.asarray(v, np.float32)).astype(np.float16)
    bxv[:, 128] = 1.0
    boot_xv = np.ascontiguousarray(
        bxv.reshape(2 * CH // 128, 128, 130).transpose(1, 0, 2)
    )
    in_maps = []
    for c in range(NCORES):
        r0 = c * RB
        xqt = np.ascontiguousarray(
            x[:, r0 : r0 + RB, :].transpose(0, 2, 1)
        ).astype(np.float16)
        maskT = np.ascontiguousarray(A_shape[r0 : r0 + RB, :].T, dtype=np.float16)
        boot_qt = np.ascontiguousarray(
            (x[0, r0 : r0 + RB, :] @ q32).T
        ).astype(np.float16)
        in_maps.append(
            {
                "xt": xt,
                "xqt": xqt,
                "maskT": maskT,
                "q_hi": q_hi,
                "q_lo": q_lo,
                "k_hi": k_hi,
                "k_lo": k_lo,
                "v": v16,
                "boot_qt": boot_qt,
                "boot_kt": boot_kt,
                "boot_xv": boot_xv,
            }
        )
    return in_maps


def kernel(x, A_shape, q, k, v):
    nc = _get_program()
    in_maps = make_in_maps(x, A_shape, q, k, v)
    res = run_bass_kernel_spmd(nc, in_maps, list(range(NCORES)))
    out = np.concatenate([res.results[c]["out"] for c in range(NCORES)], axis=1)
    return out.astype(np.float32)



# revision 47
# speedup vs baseline: 1.4753x; 1.0047x over previous
"""Trainium2 Bass kernel for a DP-GAT layer (dense masked attention).

Computes, for x:[B,N,D], A_shape:[N,N] (0/1 adjacency), q,k,v:[D,D]:
    Q = x@q ; K = x@k
    S = Q @ K^T / sqrt(D)
    W = exp(8*tanh(S/8)) * A_shape
    out = (W / W.sum(-1, keepdims=True)) @ x @ v

Sharding: rows of N split across 8 NeuronCores (1024 rows each), SPMD,
no collectives. Each core streams its row-block of the mask, computes
scores in a flash-attention-style fused loop, and writes its row-block
of the output. Host scatters inputs / gathers outputs.

Numerics: q,k are split on the host into fp16 hi+lo pairs; K^T and Q^T
are computed as two-pass fp16 matmuls with fp32 PSUM accumulation and
stored as fp16. fp16 score operands keep the final output within ~2e-3
relative of the fp32 reference (fp16 matmuls run at full PE rate, and
the exp(8*tanh) amplification of coarser dtypes is unacceptable).

Device-side flow (per core, per batch):
    KT  = k^T @ x^T  (fp16 2-pass)   [D, N]
    QT  = q^T @ xrows^T (fp16 2-pass)[D, RB]
    xv  = x @ v (+ ones col)         [N, D+1] fp16
    per i-chunk of 512 query rows:
      per group of 4 key-tiles (512 keys):
        S^T  = KT_tile^T @ QT_chunk      -> PSUM [128, 4, 512] fp32
        u    = tanh(S^T / (8*sqrt(D)))   -> SBUF fp32  (ScalarE, scale fused)
        w    = exp(8*u)                  -> SBUF fp16  (ScalarE, scale fused)
        p    = w * maskT_tile            -> SBUF fp16  (VectorE)
        acc[i,0:129] += p_slice^T @ xv   -> PSUM       (fp16 matmuls; col 128
                                                        = rowsum via ones col)
      out = acc[:, :128] * (1/acc[:, 128])  -> DMA to DRAM

The kernel is ScalarE(ACT)-bound: tanh and exp must each touch every
score element (33.6M per core) and only the ACT engine has
transcendentals, so the floor is ~2 x 262k cycles/partition (~500us).
The main loop is a 3-deep software pipeline keeping ACT at ~98% duty
(see the comment in the loop): tanh_g leads its period so the next
group's score matmuls (WAR on the single-buffered score PSUM) release
~2us before tanh_{g+1} needs them, and the PV stage lags 3 iterations
so its DVE-produced operand is always ready and the end-of-chunk
normalize overlaps the next chunk.

The per-batch prep (KT/QT/xv) is interleaved between groups (batch b+1
prep inside batch b's last i-chunk; batch 0's prep between its own
first-i-chunk groups). Besides overlap, the prep matmuls and the fp16
mask streaming keep the PE/DMA demand high enough that the chip's
demand-following DVFS governor (ham k=4 windows, visible in the NTFF
profile) holds full clocks: variants that precomputed the projections
on the host and/or shrank mask traffic to fp8 dropped the PE (and
sometimes ACT) into a ~half-rate state and ran 100-250us SLOWER despite
doing less work.

PSUM bank budget (8 banks of 2KB): score group 4 + PV accumulator 2 +
prep 2. PE matmuls with start=True clear their entire output PSUM bank,
so the two acc slots sharing a bank are zeroed by one full-bank dummy
matmul and all PV matmuls accumulate with start=False. The accumulator
is single-buffered but copied to SBUF in one DVE op before the
normalize, so it frees within ~1 period of its last PV.
"""

import math
import sys
from contextlib import ExitStack

import numpy as np

try:
    import concourse.bass as bass  # noqa: F401
except ImportError:  # pragma: no cover
    sys.path.insert(0, "/opt/trn_rl_repo")
    import concourse.bass as bass  # noqa: F401

import concourse.mybir as mybir
import concourse.tile as tile
from concourse import bacc
from concourse.bass_utils import run_bass_kernel_spmd

F32 = mybir.dt.float32
F16 = mybir.dt.float16

B, N, D = 4, 8192, 128
NCORES = 8
RB = N // NCORES  # query rows per core

IC = 512          # query-row chunk (free dim of score matmuls)
NIC = RB // IC    # i-chunks per core
JG = 4            # key 128-tiles per score group
NJT = N // 128    # key tiles total
NG = NJT // JG    # groups per i-chunk
CH = JG * 128     # xt prep chunk width (chunk g produces what group g consumes)


def build_program():
    nc = bacc.Bacc("TRN2", target_bir_lowering=False, debug=False)

    xt = nc.dram_tensor("xt", [B, D, N], F16, kind="ExternalInput").ap()
    xqt = nc.dram_tensor("xqt", [B, D, RB], F16, kind="ExternalInput").ap()
    # host-precomputed batch-0 boot tensors: qt plus kt/xv for key chunks
    # 0-1, so the first score group starts ~11us earlier (device prep's
    # serial xq-DMA -> QT-matmul -> copy chain is skipped for batch 0)
    bqt_d = nc.dram_tensor("boot_qt", [D, RB], F16, kind="ExternalInput").ap()
    bkt_d = nc.dram_tensor("boot_kt", [D, 2 * CH], F16, kind="ExternalInput").ap()
    bxv_d = nc.dram_tensor("boot_xv", [128, 2 * CH // 128, 130], F16, kind="ExternalInput").ap()
    maskT = nc.dram_tensor("maskT", [N, RB], F16, kind="ExternalInput").ap()
    qh_d = nc.dram_tensor("q_hi", [D, D], F16, kind="ExternalInput").ap()
    ql_d = nc.dram_tensor("q_lo", [D, D], F16, kind="ExternalInput").ap()
    kh_d = nc.dram_tensor("k_hi", [D, D], F16, kind="ExternalInput").ap()
    kl_d = nc.dram_tensor("k_lo", [D, D], F16, kind="ExternalInput").ap()
    v_d = nc.dram_tensor("v", [D, D], F16, kind="ExternalInput").ap()
    out_d = nc.dram_tensor("out", [B, RB, D], F32, kind="ExternalOutput").ap()

    # [128, key-tile, query-col] view of the transposed mask block
    maskT_r = maskT.rearrange("(t p) i -> p t i", p=128)

    tanh_scale = 1.0 / (8.0 * math.sqrt(float(D)))

    with tile.TileContext(nc) as tc, ExitStack() as ctx:
        consts = ctx.enter_context(tc.tile_pool(name="consts", bufs=1))
        kt_pool = ctx.enter_context(tc.tile_pool(name="kt", bufs=2))
        qt_pool = ctx.enter_context(tc.tile_pool(name="qt", bufs=2))
        xv_pool = ctx.enter_context(tc.tile_pool(name="xv", bufs=2))
        xc_pool = ctx.enter_context(tc.tile_pool(name="xc", bufs=3))
        m_pool = ctx.enter_context(tc.tile_pool(name="m", bufs=4))
        u_pool = ctx.enter_context(tc.tile_pool(name="u", bufs=3))
        w_pool = ctx.enter_context(tc.tile_pool(name="w", bufs=3))
        p_pool = ctx.enter_context(tc.tile_pool(name="p", bufs=4))
        ac_pool = ctx.enter_context(tc.tile_pool(name="ac", bufs=2))
        ob_pool = ctx.enter_context(tc.tile_pool(name="ob", bufs=4))
        rs_pool = ctx.enter_context(tc.tile_pool(name="rs", bufs=4))
        prep_ps = ctx.enter_context(tc.tile_pool(name="prep_ps", bufs=2, space="PSUM"))
        st_ps = ctx.enter_context(tc.tile_pool(name="st_ps", bufs=1, space="PSUM"))
        acc_ps = ctx.enter_context(tc.tile_pool(name="acc_ps", bufs=1, space="PSUM"))

        tiles = {}  # b -> (kt, qt, xv)

        def boot_head():
            """Batch-0 tiles come from the host boot tensors: qt whole,
            kt/xv chunks 0-1 by DMA; ones column only for tiles 8+."""
            kt = kt_pool.tile([128, N], F16)
            qt = qt_pool.tile([128, RB], F16)
            xv = xv_pool.tile([128, NJT, 130], F16)
            tiles[0] = (kt, qt, xv)
            nc.sync.dma_start(qt[:], bqt_d[:])
            nc.sync.dma_start(kt[:, 0 : 2 * CH], bkt_d[:])
            nc.sync.dma_start(xv[:, 0 : 2 * CH // 128, :], bxv_d[:])
            nc.vector.memset(xv[:, 2 * CH // 128 :, 128:129], 1.0)

        def prep_head(b):
            """Allocate batch-b tiles; compute QT; set xv ones column."""
            kt = kt_pool.tile([128, N], F16)
            qt = qt_pool.tile([128, RB], F16)
            xv = xv_pool.tile([128, NJT, 130], F16)
            tiles[b] = (kt, qt, xv)
            nc.vector.memset(xv[:, :, 128:129], 1.0)
            xq = qt_pool.tile([128, RB], F16, tag="xq")
            nc.sync.dma_start(xq[:], xqt[b])
            qch = min(CH, RB)
            for c in range(RB // qch):
                pq = prep_ps.tile([128, qch], F32, tag="prep")
                nc.tensor.matmul(
                    pq[:], qh_sb[:], xq[:, c * qch : (c + 1) * qch],
                    start=True, stop=False,
                )
                nc.tensor.matmul(
                    pq[:], ql_sb[:], xq[:, c * qch : (c + 1) * qch],
                    start=False, stop=True,
                )
                nc.vector.tensor_copy(qt[:, c * qch : (c + 1) * qch], pq[:])

        def prep_chunk(b, c):
            """Compute kt columns and xv tiles for chunk c of batch b."""
            kt, _, xv = tiles[b]
            xc = xc_pool.tile([128, CH], F16)
            nc.sync.dma_start(xc[:], xt[b][:, c * CH : (c + 1) * CH])
            pk = prep_ps.tile([128, CH], F32, tag="prep")
            nc.tensor.matmul(pk[:], kh_sb[:], xc[:], start=True, stop=False)
            nc.tensor.matmul(pk[:], kl_sb[:], xc[:], start=False, stop=True)
            nc.vector.tensor_copy(kt[:, c * CH : (c + 1) * CH], pk[:])
            for s in range(CH // 128):
                pxv = prep_ps.tile([128, 128], F32, tag="prep")
                nc.tensor.matmul(
                    pxv[:], xc[:, s * 128 : (s + 1) * 128], v_sb[:],
                    start=True, stop=True,
                )
                nc.vector.tensor_copy(xv[:, c * (CH // 128) + s, 0:128], pxv[:])

        def zero_acc(acc):
            # PE start=True clears the WHOLE PSUM bank, so the two acc
            # slots sharing a bank are zeroed by one full-bank dummy
            # matmul; all real PV matmuls accumulate with start=False.
            for hb in range(2):
                nc.tensor.matmul(
                    acc[:, hb * 512 : (hb + 1) * 512],
                    zeros[:, 0:128], zeros[:],
                    start=True, stop=False, skip_group_check=True,
                )

        # ---- 3-deep software-pipelined main loop --------------------
        # stage 1 (iter gi):   score matmuls + tanh  (+ prep attachments)
        # stage 2 (iter gi+1): exp + mask multiply
        # stage 3 (iter gi+3): PV matmuls (+ zero/normalize at chunk edges)
        #
        # tanh_g is the FIRST ScalarE op of its period, so the next
        # group's score matmuls (WAR on the single-buffered score PSUM)
        # are released ~2us before tanh_{g+1} needs them, and the PE is
        # never on the ScalarE critical path. The PV stage lags by 3 so
        # its operand (p, produced by DVE ~2.3us after exp ends) is
        # always ready before the PE reaches it. The lag also slides the
        # end-of-chunk normalize into the next chunk's first iterations,
        # where it overlaps the pipeline instead of stalling it.

        SEQ = [(b, ic, g) for b in range(B) for ic in range(NIC) for g in range(NG)]
        PVLAG = 3
        st1, st2, accs = {}, {}, {}

        # boot DMAs first: the consts/zeros below are needed only by the
        # prep of later chunks/batches, and each dma_start costs ~0.6us
        # of Sync-engine issue time ahead of the first score group
        boot_head()
        zeros = consts.tile([128, 512], F16)
        nc.vector.memset(zeros[:], 0.0)
        # prewarm the ScalarE activation table (tanh+exp share one set):
        # the ~1.3us ACT_TABLE_LOAD runs concurrent with the boot DMA
        # transfers instead of serializing before the first real tanh
        warm = consts.tile([128, 1], F32)
        nc.scalar.activation(
            warm[:], zeros[:, 0:1], mybir.ActivationFunctionType.Tanh
        )
        qh_sb = consts.tile([D, D], F16)
        nc.sync.dma_start(qh_sb[:], qh_d[:])
        ql_sb = consts.tile([D, D], F16)
        nc.sync.dma_start(ql_sb[:], ql_d[:])
        kh_sb = consts.tile([D, D], F16)
        nc.sync.dma_start(kh_sb[:], kh_d[:])
        kl_sb = consts.tile([D, D], F16)
        nc.sync.dma_start(kl_sb[:], kl_d[:])
        v_sb = consts.tile([D, D], F16)
        nc.sync.dma_start(v_sb[:], v_d[:])

        for gi in range(len(SEQ) + PVLAG):
            if gi < len(SEQ):
                b, ic, g = SEQ[gi]
                if ic == NIC - 1 and g == 0 and b + 1 < B:
                    prep_head(b + 1)
                if b == 0 and ic == 0 and g + 2 < NG:
                    prep_chunk(0, g + 2)
                if ic == NIC - 1 and b + 1 < B:
                    prep_chunk(b + 1, g)
                kt, qt, xv = tiles[b]
                stp = st_ps.tile([128, JG, IC], F32)
                for j in range(JG):
                    nc.tensor.matmul(
                        stp[:, j],
                        kt[:, (g * JG + j) * 128 : (g * JG + j + 1) * 128],
                        qt[:, ic * IC : (ic + 1) * IC],
                        start=True, stop=True,
                    )
                m = m_pool.tile([128, JG, IC], F16)
                nc.sync.dma_start(
                    m[:], maskT_r[:, g * JG : (g + 1) * JG, ic * IC : (ic + 1) * IC]
                )
                u = u_pool.tile([128, JG, IC], F32)
                nc.scalar.activation(
                    u[:], stp[:], mybir.ActivationFunctionType.Tanh, scale=tanh_scale
                )
                st1[gi] = (u, m)

            j2 = gi - 1
            if 0 <= j2 < len(SEQ):
                u, m = st1.pop(j2)
                w = w_pool.tile([128, JG, IC], F16)
                nc.scalar.activation(
                    w[:], u[:], mybir.ActivationFunctionType.Exp, scale=8.0
                )
                p = p_pool.tile([128, JG, IC], F16)
                nc.vector.tensor_mul(p[:], w[:], m[:])
                st2[j2] = p

            j3 = gi - PVLAG
            if 0 <= j3 < len(SEQ):
                b, ic, g = SEQ[j3]
                if g == 0:
                    acc = acc_ps.tile([128, 1024], F32)
                    accs[(b, ic)] = acc
                    zero_acc(acc)
                acc = accs[(b, ic)]
                p = st2.pop(j3)
                xv = tiles[b][2]
                for j in range(JG):
                    for s in range(IC // 128):
                        nc.tensor.matmul(
                            acc[:, s * 256 : s * 256 + 129],
                            p[:, j, s * 128 : (s + 1) * 128],
                            xv[:, g * JG + j, 0:129],
                            start=False,
                            stop=(g == NG - 1 and j == JG - 1),
                            skip_group_check=True,
                        )
                if g == NG - 1:
                    # copy acc out of PSUM in one op so the single
                    # accumulator buffer frees quickly, then normalize
                    # from the SBUF copy: one strided reciprocal over the
                    # 4 rowsum columns, and one DMA for all 512 out rows.
                    # The last chunk has no successor waiting on acc, so
                    # it normalizes from PSUM directly.
                    if b == B - 1 and ic == NIC - 1:
                        ac = acc
                    else:
                        ac = ac_pool.tile([128, 1024], F32)
                        nc.vector.tensor_copy(ac[:], acc[:])
                    del accs[(b, ic)]
                    acr = ac[:].rearrange("p (s c) -> p s c", s=IC // 128)
                    rs = rs_pool.tile([128, IC // 128], F32)
                    nc.vector.reciprocal(rs[:], acr[:, :, 128])
                    ob = ob_pool.tile([128, IC // 128, 128], F32)
                    for s in range(IC // 128):
                        nc.vector.tensor_scalar_mul(
                            ob[:, s], acr[:, s, 0:128], rs[:, s : s + 1]
                        )
                    nc.sync.dma_start(
                        out_d[b][ic * IC : (ic + 1) * IC, :].rearrange(
                            "(s q) d -> q s d", q=128
                        ),
                        ob[:],
                    )

    nc.compile()
    return nc


_CACHED_NC = None


def _get_program():
    global _CACHED_NC
    if _CACHED_NC is None:
        _CACHED_NC = build_program()
    return _CACHED_NC


def _split16(a):
    hi = a.astype(np.float16)
    lo = (a - hi.astype(np.float32)).astype(np.float16)
    return hi, lo


def make_in_maps(x, A_shape, q, k, v):
    x = np.ascontiguousarray(x, dtype=np.float32)
    xt = np.ascontiguousarray(x.transpose(0, 2, 1)).astype(np.float16)  # [B, D, N]
    q_hi, q_lo = _split16(np.ascontiguousarray(q, dtype=np.float32))
    k_hi, k_lo = _split16(np.ascontiguousarray(k, dtype=np.float32))
    v16 = np.ascontiguousarray(v, dtype=np.float32).astype(np.float16)
    q32 = np.asarray(q, np.float32)
    # batch-0 boot tensors (see build_program): kt/xv for keys 0:2*CH
    boot_kt = np.ascontiguousarray(
        (x[0, : 2 * CH, :] @ np.asarray(k, np.float32)).T
    ).astype(np.float16)
    bxv = np.zeros((2 * CH, 130), np.float16)
    bxv[:, :128] = (x[0, : 2 * CH, :] @ np.asarray(v, np.float32)).astype(np.float16)
    bxv[:, 128] = 1.0
    boot_xv = np.ascontiguousarray(
        bxv.reshape(2 * CH // 128, 128, 130).transpose(1, 0, 2)
    )
    in_maps = []
    for c in range(NCORES):
        r0 = c * RB
        xqt = np.ascontiguousarray(
            x[:, r0 : r0 + RB, :].transpose(0, 2, 1)
        ).astype(np.float16)
        maskT = np.ascontiguousarray(A_shape[r0 : r0 + RB, :].T, dtype=np.float16)
        boot_qt = np.ascontiguousarray(
            (x[0, r0 : r0 + RB, :] @ q32).T
        ).astype(np.float16)
        in_maps.append(
            {
                "xt": xt,
                "xqt": xqt,
                "maskT": maskT,
                "q_hi": q_hi,
                "q_lo": q_lo,
                "k_hi": k_hi,
                "k_lo": k_lo,
                "v": v16,
                "boot_qt": boot_qt,
                "boot_kt": boot_kt,
                "boot_xv": boot_xv,
            }
        )
    return in_maps


def kernel(x, A_shape, q, k, v):
    nc = _get_program()
    in_maps = make_in_maps(x, A_shape, q, k, v)
    res = run_bass_kernel_spmd(nc, in_maps, list(range(NCORES)))
    out = np.concatenate([res.results[c]["out"] for c in range(NCORES)], axis=1)
    return out.astype(np.float32)

